# revision 28
# baseline (speedup 1.0000x reference)
"""Quantum angle-encoder state-vector kernel for Trainium2 (8 NeuronCores).

For each batch row b and qubit q the gate rz*ry applied to |0> contributes a
2-vector col0 = cos(ry/2)e^{-i rz/2}, col1 = sin(ry/2)e^{+i rz/2}; the output
state is the Kronecker product over 16 qubits (qubit 0 = MSB), [B, 2^16] c64.

Per core (32 batch rows, pure data parallel over 8 cores) the HBM write of the
[32, 65536] c64 output (16.8 MB) is the roofline (~43.5us at the ~387 GB/s
multi-ring HBM write rate measured on this part), so the design minimizes the
serial head before the output stream starts and keeps both HWDGE rings busy:

  * v = v_hi (x) v_lo with v_hi/v_lo the 8-qubit half-products (length 256),
    built in SIGNED-polar form stacked on 64 partitions (rows 0:32 hi,
    32:64 lo): magnitude = product of signed cos/sin(ry/2) terms (signs fold
    into the magnitude, so no abs/mask/pi-phase ops), phase = sum of -/+ rz/2.
  * ONE [128,8] input DMA: host pre-stacks ry on partitions 0:64 and rz on
    64:128 ([BC,16] -> [2,BC,8] per tensor).
  * Phases are additive -> ONE K=16 bf16 selection matmul with +-0.5
    weights applied to the split-bf16 (h+l) of the RAW rz angles (~1e-4
    absolute accuracy).
  * Magnitudes multiply -> doubling chain split per level between ACT
    (t=0 block, activation-Identity with a per-partition scale column) and
    DVE (t=1 block, tensor_scalar with a column operand); the LAST level is
    folded into the vr/vi-producing STTs ((cos*c0col)*mag128).
  * The ONLY activation function used anywhere is Sin: one table load,
    prefetched by a dummy activation first on the ACT queue (inputs ride
    the Sync ring so nothing queues behind the ~1.3us table load). Range
    reduction in "turns" via the DVE f32->i32 rounding cast; cos/sin =
    Sin(2pi d + pi/2) / Sin(2pi d), args inside the LUT's accurate domain
    (HW-probed: ~1e-7 to |x|<=3.1, ~5e-5 out to 3.5 which covers the
    unreduced cos(ry/2) args).
  * The 256x256 outer product per row is a K=2 bf16 matmul (tolerance
    2e-2; bf16 factors give ~2e-3): rhs columns pre-interleaved so PSUM
    comes out in complex64 memory order. Two matmuls per batch row with
    STRIDE-2 lhsT views (even/odd output indices i=2p / i=2p+1); the two
    PSUM tiles are copied side by side (Vector/Scalar alternate) into
    staging SBUF tiles forming 4KB-contiguous HBM lines. Output DMAs:
    single rows 0-3 for a fast ramp, then 1MB row-PAIRS, ALTERNATING
    between the Sync and Scalar HWDGE rings (adding the SWDGE ring drops
    the aggregate rate via descriptor-ring port contention). Every DMA's
    completion retires 16 HBM-receipt-gated semaphore increments whose
    ~2us round trips can stall the rings when they bunch up at the end of
    the stream -- halving the DMA count halves that exposure (a 1MB DMA's
    2.6us of data time hides its receipt; a 512KB one's 1.3us does not).
  * The matmul operands need the factors flattened to partitions 0/1
    (partition-base rule) -> SBUF->SBUF DMAs (rh on Sync, lh on GpSimd),
    split into a small first chunk (rows 0:CH) in SEPARATE tiles from the
    rest, so the first row's matmul does not wait on the big flatten.

Notes for this toolchain: walrus here encodes at most ONE semaphore wait per
instruction -- _legalize_single_wait() hoists extra Tile-emitted waits into
standalone EventSemaphore instructions. Matmul operands must sit at SBUF
partition base 0/32/64, hence the flattening DMAs. GpSimd has no PSUM port
and is 4-8x slower than DVE/ACT on bulk elementwise -- it only issues DMAs
and memsets here. Output per core [32,128,1024] f32 == [32, 65536] complex64
(viewed on host).
"""

import os

import numpy as np
import ml_dtypes

import concourse.bass as bass
import concourse.mybir as mybir
import concourse.tile as tile
from concourse.bass_utils import run_bass_kernel_spmd

N_CORES = 8
B, Q = 256, 16
BC = B // N_CORES  # batch rows per core
HQ = Q // 2  # qubits per half
HL = 1 << HQ  # 256: length of each half-product
P2 = 2 * BC  # 64: both halves stacked on partitions
CH = 4  # rows in the early flatten chunk
F32 = mybir.dt.float32
BF16 = mybir.dt.bfloat16
I32 = mybir.dt.int32
PI_HALF = float(np.pi / 2)
TWO_PI = float(2.0 * np.pi)
INV2PI = float(1.0 / (2.0 * np.pi))

_AF = mybir.ActivationFunctionType
_OP = mybir.AluOpType


def _legalize_single_wait(nc):
    """This walrus build encodes at most one semaphore wait per instruction
    ("Too many sync wait commands" otherwise). Hoist extra waits into
    standalone EventSemaphore instructions placed immediately before — a
    sequencer-level wait gates everything after it on the same engine, so
    semantics are preserved (slightly stronger ordering)."""
    cnt = 0
    for fn in nc.m.functions:
        for blk in fn.blocks:
            out = []
            for ins in blk.instructions:
                si = ins.sync_info
                if si is not None and si.on_wait is not None and len(si.on_wait) > 1:
                    waits = list(si.on_wait)
                    for w in waits[:-1]:
                        cnt += 1
                        ev = mybir.InstEventSemaphore(
                            name=f"{ins.name}-presync-{cnt}", ins=[], outs=[]
                        )
                        ev.engine = ins.engine
                        ev.sync_info = mybir.SyncInfo(on_wait=[w], on_update=[])
                        out.append(ev)
                    ins.sync_info = mybir.SyncInfo(
                        on_wait=[waits[-1]], on_update=list(si.on_update)
                    )
                out.append(ins)
            try:
                blk.instructions = out
            except Exception:
                blk.instructions[:] = out
    return cnt


def _sel_matrix():
    """[16, 256] phase-selection matrix, rows = (h/l split 2) x (q 8), value
    +-0.5 by the qubit's bit of the half-index (qubit col 0 = MSB). th[b,j] =
    sum_q sign(bit_q(j)) * 0.5 * rz[b,q] with rz = h + l split-bf16."""
    sel = np.zeros((16, HL), dtype=np.float32)
    j = np.arange(HL)
    for g in range(2):
        for q in range(HQ):
            bits = (j >> (HQ - 1 - q)) & 1
            sel[g * HQ + q, :] = np.where(bits == 1, 0.5, -0.5)
    return sel.astype(ml_dtypes.bfloat16)


def build_bass():
    nc = bass.Bass()
    # Host pre-stacks each [BC, 16] input as [2, BC, 8] -> ry on partitions
    # 0:64, rz on 64:128 (hi qubits 0:8 on the first 32 rows of each group).
    ang_d = nc.dram_tensor("ang", [2 * P2, HQ], F32, kind="ExternalInput")
    # Partition p of a row's staging tile holds output indices i=2p (cols
    # 0:512) and i=2p+1 (cols 512:1024) -> every output DMA line is 4KB
    # contiguous in HBM ([bi, p] block = i 2p..2p+1, all j).
    out_d = nc.dram_tensor("out", [BC, 128, 1024], F32, kind="ExternalOutput")

    # sel + ident packed in ONE [128, 320] bf16 const: sel on partitions
    # 0:16 cols 0:256, ident on partitions 64:128 cols 256:320 (base 64 to
    # match ph2 for the transpose's same-base-partition rule)
    cst = np.zeros((2 * P2, HL + P2), dtype=ml_dtypes.bfloat16)
    cst[0:16, 0:HL] = _sel_matrix()
    cst[P2 : 2 * P2, HL : HL + P2] = np.eye(P2).astype(ml_dtypes.bfloat16)
    cst_d = nc.inline_tensor(cst, name="cst_const")

    with tile.TileContext(nc) as tc:
        with (
            tc.tile_pool(name="io", bufs=1) as io,
            tc.tile_pool(name="stage", bufs=8) as stage,
            tc.tile_pool(name="psum", bufs=8, space="PSUM") as psum,
        ):
            # ---- ACT: Sin-table prefetch FIRST on its queue ----------------
            warm = io.tile([P2, 1], F32, tag="warm")
            nc.gpsimd.memset(warm[:], 0.25)
            pih = io.tile([P2, 1], F32, tag="pih")
            nc.gpsimd.memset(pih[:], PI_HALF)
            wo = io.tile([P2, 1], F32, tag="wo")
            nc.scalar.activation(wo[:], warm[:], _AF.Sin)

            # ---- input then consts, both on the Sync ring ------------------
            ang = io.tile([2 * P2, HQ], F32, tag="ang")
            nc.sync.dma_start(ang[:], ang_d[:])
            sry = ang[0:P2, :]
            srz = ang[P2 : 2 * P2, :]
            cstt = io.tile([2 * P2, HL + P2], BF16, tag="cst")
            nc.sync.dma_start(cstt[:], cst_d[:])
            sel = cstt[0:16, 0:HL]
            ident = cstt[P2 : 2 * P2, HL : HL + P2]  # base 64, matches ph2

            # ---- split-bf16 of rz (ACT h-copy, DVE residual) ---------------
            ph2t = io.tile([2 * P2, 16], BF16, tag="ph2")
            ph2 = ph2t[P2 : 2 * P2, :]
            nc.scalar.copy(ph2[:, 0:HQ], srz)
            # residual straight to bf16: tensor_sub in f32 with RNE bf16 out
            nc.vector.tensor_sub(ph2[:, HQ:16], srz, ph2[:, 0:HQ])

            # ---- cs8 = [cos(ry/2) | sin(ry/2)], SIGNED (args probed safe) --
            # scale/bias fused into the Sin activations: cos = Sin(0.5ry+pi/2)
            cs8 = io.tile([P2, 16], F32, tag="cs8")
            nc.scalar.activation(
                cs8[:, 0:HQ], sry, _AF.Sin, bias=pih[:], scale=0.5
            )
            nc.scalar.activation(cs8[:, HQ:16], sry, _AF.Sin, scale=0.5)

            # ---- one transpose + one K=16 selection matmul -----------------
            tp = psum.tile([16, P2], BF16, tag="tp", bufs=1)
            nc.tensor.transpose(tp[:], ph2, ident)
            vals = io.tile([16, P2], BF16, tag="vals")
            nc.vector.tensor_copy(vals[:], tp[:])
            th = psum.tile([P2, HL], F32, tag="th", bufs=1)
            nc.tensor.matmul(th[:], vals[:], sel, start=True, stop=True)

            # ---- signed magnitude chain: ACT t=0 block, DVE t=1 block ------
            # 6 levels (q=6..1); the q=0 level is folded into the vr/vi STTs.
            mA = io.tile([P2, 128], F32, tag="mA")
            mB = io.tile([P2, 128], F32, tag="mB")
            nc.gpsimd.tensor_copy(mA[:, 0:1], cs8[:, HQ - 1 : HQ])
            nc.vector.tensor_copy(mA[:, 1:2], cs8[:, 15:16])
            chain = []  # (dst, src, q, L)
            cur, nxt = mA, mB
            L = 2
            for q in range(HQ - 2, 0, -1):
                chain.append((nxt, cur, q, L))
                cur, nxt = nxt, cur
                L *= 2
            mag128 = cur  # [64, 128] products of qubits 1..7

            def t0(step):
                # t=0 blocks ride the otherwise-idle GpSimd (slow but off
                # the critical path for ln<=32); the last level's t=0 runs
                # on DVE (GP would take ~850ns at ln=64). ACT is kept free
                # for the Sin calls — anything queued before them on the
                # ACT ring delays cs_cos/cs_sin by program order.
                dst, src, q, ln = step
                eng = nc.vector if ln >= 64 else nc.gpsimd
                eng.tensor_scalar(
                    dst[:, 0:ln], src[:, 0:ln],
                    cs8[:, q : q + 1], None, op0=_OP.mult,
                )

            def t1(step):
                dst, src, q, ln = step
                nc.vector.tensor_scalar(
                    dst[:, ln : 2 * ln], src[:, 0:ln],
                    cs8[:, HQ + q : HQ + q + 1], None, op0=_OP.mult,
                )

            # ---- range-reduced cos/sin on DVE, interleaved with the chain --
            # NOTE: Tile dependency tracking follows EMISSION order — a
            # reader emitted before its writer silently reads stale data.
            # Each level's t0 (ACT) + t1 (DVE) are emitted together; the
            # engines still overlap via the Tile-inserted semaphores.
            ni = io.tile([P2, 512], I32, tag="ni")
            nf = io.tile([P2, 512], F32, tag="nf")
            dd = io.tile([P2, 512], F32, tag="dd")
            t0(chain[0])
            t1(chain[0])
            t0(chain[1])
            t1(chain[1])
            nc.vector.tensor_scalar(
                ni[:, 0:HL], th[:, 0:HL], INV2PI, 0.25, op0=_OP.mult, op1=_OP.add
            )
            t0(chain[2])
            t1(chain[2])
            nc.vector.tensor_copy(nf[:, 0:HL], ni[:, 0:HL])
            nc.vector.scalar_tensor_tensor(
                dd[:, 0:HL], th[:, 0:HL], INV2PI, nf[:, 0:HL],
                op0=_OP.mult, op1=_OP.subtract,
            )
            t0(chain[3])
            t1(chain[3])
            nc.vector.tensor_scalar(
                ni[:, HL : 2 * HL], th[:, 0:HL], INV2PI, None, op0=_OP.mult
            )
            t0(chain[4])
            t1(chain[4])
            nc.vector.tensor_copy(nf[:, HL : 2 * HL], ni[:, HL : 2 * HL])
            nc.vector.scalar_tensor_tensor(
                dd[:, HL : 2 * HL], th[:, 0:HL], INV2PI, nf[:, HL : 2 * HL],
                op0=_OP.mult, op1=_OP.subtract,
            )
            t0(chain[5])
            t1(chain[5])
            cs = io.tile([P2, 512], F32, tag="cs")
            # cols 0:256 = cos(theta) = sin(2pi d_c + pi/2), 256:512 = sin
            nc.scalar.activation(
                cs[:, 0:HL], dd[:, 0:HL], _AF.Sin, bias=pih[:], scale=TWO_PI
            )
            nc.scalar.activation(
                cs[:, HL : 2 * HL], dd[:, HL : 2 * HL], _AF.Sin, scale=TWO_PI
            )

            # ---- bf16 factors with the q=0 chain level folded in -----------
            # vr[j] = cos[j]*mag128[j%128]*(c0 if j<128 else s0), vi likewise
            c0 = cs8[:, 0:1]
            s0 = cs8[:, HQ : HQ + 1]
            vr = io.tile([P2, HL], BF16, tag="vr")
            vi = io.tile([P2, HL], BF16, tag="vi")
            nc.vector.scalar_tensor_tensor(
                vr[:, 0:128], cs[:, 0:128], c0, mag128[:], op0=_OP.mult, op1=_OP.mult
            )
            nc.vector.scalar_tensor_tensor(
                vr[:, 128:256], cs[:, 128:256], s0, mag128[:],
                op0=_OP.mult, op1=_OP.mult,
            )
            nc.vector.scalar_tensor_tensor(
                vi[:, 0:128], cs[:, 256:384], c0, mag128[:],
                op0=_OP.mult, op1=_OP.mult,
            )
            nc.vector.scalar_tensor_tensor(
                vi[:, 128:256], cs[:, 384:512], s0, mag128[:],
                op0=_OP.mult, op1=_OP.mult,
            )
            # lo half (partitions 32:64): interleaved rhs patterns.
            #   PT1 = interleave(lr, ll), PT2 = interleave(-ll, lr)
            pt1 = io.tile([P2, 2 * HL], BF16, tag="pt1")
            v1 = pt1[BC:P2, :].rearrange("p (j t) -> p j t", t=2)
            pt2 = io.tile([P2, 2 * HL], BF16, tag="pt2")
            v2 = pt2[BC:P2, :].rearrange("p (j t) -> p j t", t=2)
            # DVE strided copies are ~2x faster than ACT's (302 vs 577ns);
            # ACT takes only the vr->pt2 copy, overlapped after cs_sin
            nc.scalar.copy(v2[:, :, 1], vr[BC:P2, :])
            nc.vector.tensor_copy(v1[:, :, 0], vr[BC:P2, :])
            nc.vector.tensor_copy(v1[:, :, 1], vi[BC:P2, :])
            nc.vector.tensor_scalar(
                v2[:, :, 0], vi[BC:P2, :], -1.0, None, op0=_OP.mult
            )

            # ---- flatten to matmul operands: small chunk A first -----------
            # (separate tiles so row<CH matmuls depend only on chunk A;
            #  rh on the Sync ring, lh on the GpSimd SWDGE ring)
            rhA = io.tile([2, CH * 2 * HL], BF16, tag="rhA")
            lhA = io.tile([2, CH * HL], BF16, tag="lhA")
            nc.gpsimd.dma_start(lhA[0:1, :], vr[0:CH, :])
            nc.sync.dma_start(rhA[0:1, :], pt1[BC : BC + CH, :])
            nc.gpsimd.dma_start(lhA[1:2, :], vi[0:CH, :])
            nc.sync.dma_start(rhA[1:2, :], pt2[BC : BC + CH, :])
            RB = BC - CH
            rhB = io.tile([2, RB * 2 * HL], BF16, tag="rhB")
            lhB = io.tile([2, RB * HL], BF16, tag="lhB")
            nc.gpsimd.dma_start(lhB[0:1, :], vr[CH:BC, :])
            nc.sync.dma_start(rhB[0:1, :], pt1[BC + CH : P2, :])
            nc.gpsimd.dma_start(lhB[1:2, :], vi[CH:BC, :])
            nc.sync.dma_start(rhB[1:2, :], pt2[BC + CH : P2, :])

            # ---- main loop: 2 K=2 matmuls + 2 copies per row ---------------
            # Output DMAs: single rows 0-3 (fast ramp), then 1MB row-PAIRS.
            # Every DMA completion retires 16 HBM-receipt-gated semaphore
            # increments at the end of the stream (~1.3-1.9us each lane);
            # halving the DMA count halves that drain tail. Two HWDGE rings
            # alternate (adding the SWDGE ring degrades the aggregate via
            # descriptor-ring port contention, so GpSimd only issues
            # pre-stream DMAs).
            ot2 = None
            for bi in range(BC):
                if bi < CH:
                    rhs = rhA[:, bi * 2 * HL : (bi + 1) * 2 * HL]
                    lhr = lhA[:, bi * HL : (bi + 1) * HL]
                else:
                    bj = bi - CH
                    rhs = rhB[:, bj * 2 * HL : (bj + 1) * 2 * HL]
                    lhr = lhB[:, bj * HL : (bj + 1) * HL]
                # stride-2 views: even hi values feed partitions p -> i=2p,
                # odd -> i=2p+1
                lhv = lhr.rearrange("k (i e) -> k i e", e=2)
                acc_e = psum.tile([128, 512], F32, tag="acc", bufs=6)
                nc.tensor.matmul(acc_e[:], lhv[:, :, 0], rhs, start=True, stop=True)
                acc_o = psum.tile([128, 512], F32, tag="acc", bufs=6)
                nc.tensor.matmul(acc_o[:], lhv[:, :, 1], rhs, start=True, stop=True)
                paired = CH <= bi < BC - 2
                if paired:
                    if (bi - CH) % 2 == 0:
                        ot2 = stage.tile([128, 2048], F32, tag="ot2")
                    ot = ot2
                    off = ((bi - CH) % 2) * 1024
                else:
                    ot = stage.tile([128, 1024], F32, tag="ot")
                    off = 0
                if bi % 2 == 0:
                    nc.vector.tensor_copy(ot[:, off : off + 512], acc_e[:])
                    nc.scalar.copy(ot[:, off + 512 : off + 1024], acc_o[:])
                else:
                    nc.scalar.copy(ot[:, off : off + 512], acc_e[:])
                    nc.vector.tensor_copy(ot[:, off + 512 : off + 1024], acc_o[:])
                if bi == 0:
                    # split row 0 across both rings: earliest first bytes
                    nc.sync.dma_start(out_d[0, :, 0:512], ot[:, 0:512])
                    nc.scalar.dma_start(out_d[0, :, 512:1024], ot[:, 512:1024])
                elif bi < CH:
                    (nc.sync if bi % 2 == 1 else nc.scalar).dma_start(
                        out_d[bi], ot[:]
                    )
                elif paired and (bi - CH) % 2 == 1:
                    dst = out_d[bi - 1 : bi + 1].rearrange("r p c -> p r c")
                    src = ot[:].rearrange("p (r c) -> p r c", r=2)
                    (nc.sync if ((bi - CH) // 2) % 2 == 0 else nc.scalar).dma_start(
                        dst, src
                    )
                elif bi == BC - 2:
                    nc.scalar.dma_start(out_d[bi], ot[:])
                elif bi == BC - 1:
                    # split the final row across both rings: short drain
                    nc.sync.dma_start(out_d[bi, :, 0:512], ot[:, 0:512])
                    nc.scalar.dma_start(out_d[bi, :, 512:1024], ot[:, 512:1024])
    _legalize_single_wait(nc)
    return nc


_nc_cache = None


def _get_nc():
    global _nc_cache
    if _nc_cache is None:
        _nc_cache = build_bass()
    return _nc_cache


def _stack(a):
    """[BC, 16] -> [64, 8]: hi qubits (cols 0:8) on rows 0:32, lo on 32:64."""
    return a.reshape(BC, 2, HQ).transpose(1, 0, 2).reshape(P2, HQ)


def run(ry_angles, rz_angles, trace=False):
    """Shard over 8 cores, run, gather. Returns (out [B, 2**Q] c64, results)."""
    ry = np.ascontiguousarray(np.asarray(ry_angles, dtype=np.float32))
    rz = np.ascontiguousarray(np.asarray(rz_angles, dtype=np.float32))
    assert ry.shape == (B, Q) and rz.shape == (B, Q)
    nc = _get_nc()
    in_maps = [
        {
            "ang": np.ascontiguousarray(
                np.concatenate(
                    [
                        _stack(ry[k * BC : (k + 1) * BC]),
                        _stack(rz[k * BC : (k + 1) * BC]),
                    ],
                    axis=0,
                )
            )
        }
        for k in range(N_CORES)
    ]
    res = run_bass_kernel_spmd(nc, in_maps, list(range(N_CORES)), trace=trace)
    parts = [
        np.ascontiguousarray(r["out"]).reshape(BC, 2 * (1 << Q)).view(np.complex64)
        for r in res.results
    ]
    return np.concatenate(parts, axis=0), res


def kernel(ry_angles, rz_angles):
    out, _ = run(ry_angles, rz_angles, trace=False)
    return out
